# revision 29
# baseline (speedup 1.0000x reference)
"""ExpanderLinear on 8 TRN2 NeuronCores.

y = x @ (weight * mask)^T + bias
  x      [8192, 4096] f32
  weight [4096, 4096] f32
  mask   [4096, 4096] i32 (0/1)
  bias   [4096]       f32
  y      [8192, 4096] f32

Sharding: 2D 2x4 grid — 2 token shards x 4 outdim shards. Core k handles
tokens [4096*(k//4), +4096) and outdim [1024*(k%4), +1024).

Per-core dataflow (all compute on device). Measured DMA rates drive the
design: plain HWDGE loads ~330GB/s, cast-DMAs only ~230GB/s, xbar
transposes ~130GB/s under load. So:
  - NO cast DMAs: w/m/x load plain (HWDGE rings); DVE does i32->f32
    conversion implicitly in the mask multiply and the x f32->bf16 casts.
  - wm prep: w,m stream on the sync ring; DVE multiply -> bf16; the PE
    transposes wm chunks (identity matmul -> PSUM, DVE copyback) into the
    resident wmT [128, 8ot, 32kt, 128o] bf16 buffer — this rides the
    otherwise-idle ramp phase.
  - x panels [128, 32kt, 128t]: first NPE panels transposed by the PE
    (ramp); the rest via xbar dma_start_transpose (scalar ring) whose
    steady-state demand (~63GB/s) is far below its capability.
  - PE main work: per (tp, oq) a [128t, 512o] PSUM tile accumulates 32
    k-tile matmuls (lhsT = xT panel slice, rhs = 3D wmT slice).
  - DVE eviction adds bias while copying PSUM -> SBUF; y stores on SWDGE.
Ramp: the first R token tiles emit oq0 sweeps before any oq1 work, and wm
chunks stream kh-major, so PSUM accumulation starts k-tile-progressively
as soon as the first quarter of wmT has landed.
"""
import os
import sys

sys.path.insert(0, "/opt/trn_rl_repo")

import numpy as np  # noqa: E402

import concourse.bass as bass  # noqa: E402,F401
import concourse.mybir as mybir  # noqa: E402
import concourse.tile as tile  # noqa: E402
import concourse.bacc as bacc  # noqa: E402
from concourse.bass_utils import run_bass_kernel_spmd  # noqa: E402
from concourse.bass_interp import get_hw_module  # noqa: E402

TOKENS, INDIM, OUTDIM = 8192, 4096, 4096
R_SHARDS, C_SHARDS = 2, 4
T_C, O_C = TOKENS // R_SHARDS, OUTDIM // C_SHARDS  # 4096, 1024

P = 128     # partition / tile edge
OGW = 512   # psum free width (outs per accumulation group)


def build_program(t_c=T_C, o_c=O_C, k=INDIM, hw=True):
    from concourse.masks import make_identity

    TP = t_c // P            # token tiles
    OT = o_c // P            # out tiles
    KT = k // P              # k tiles
    OQ = o_c // OGW          # out groups per token tile
    OTG = OGW // P           # out tiles per group (4)
    KH = 4                   # k chunks for wm staging
    KC = k // KH
    KCT = KC // P            # k tiles per chunk (8)
    XC = 2                   # x load chunks per panel
    XKC = k // XC
    NPAN = 5                 # xT panel bufs
    R = min(4, TP)           # ramp sweeps with deferred oq1
    NPE = min(6, TP)         # panels transposed on the PE during ramp

    nc = bacc.Bacc("TRN2", target_bir_lowering=False, debug=False,
                   num_devices=8)
    x = nc.dram_tensor("x", [t_c, k], mybir.dt.float32, kind="ExternalInput")
    w = nc.dram_tensor("w", [o_c, k], mybir.dt.float32, kind="ExternalInput")
    m = nc.dram_tensor("m", [o_c, k], mybir.dt.int32, kind="ExternalInput")
    b = nc.dram_tensor("b", [o_c], mybir.dt.float32, kind="ExternalInput")
    y = nc.dram_tensor("y", [t_c, o_c], mybir.dt.float32,
                       kind="ExternalOutput")

    with tile.TileContext(nc) as tc:
        with (tc.tile_pool(name="wmT_pool", bufs=1) as wmT_pool,
              tc.tile_pool(name="biasp", bufs=1) as biasp,
              tc.tile_pool(name="consts", bufs=1) as consts,
              tc.tile_pool(name="xTp", bufs=NPAN) as xTp_pool,
              tc.tile_pool(name="wld", bufs=3) as wld_pool,
              tc.tile_pool(name="mld", bufs=3) as mld_pool,
              tc.tile_pool(name="wmbf", bufs=2) as wmbf_pool,
              tc.tile_pool(name="xld", bufs=6) as xld_pool,
              tc.tile_pool(name="xbf", bufs=2) as xbf_pool,
              tc.tile_pool(name="outp", bufs=1) as outp,
              tc.tile_pool(name="psum", bufs=1, space="PSUM") as psum_pool,
              tc.tile_pool(name="ptr", bufs=2, space="PSUM") as ptr_pool):
            wmT = wmT_pool.tile([P, OT, KT, P], mybir.dt.bfloat16, name="wmT")
            bias_bf = biasp.tile([1, o_c], mybir.dt.bfloat16, name="bias")
            nc.gpsimd.dma_start(bias_bf[:, :], b[None, :])
            ident = consts.tile([P, P], mybir.dt.bfloat16, name="ident")
            make_identity(nc, ident)
            ones_t = consts.tile([1, P], mybir.dt.bfloat16, name="ones_t")
            nc.vector.memset(ones_t[:, :], 1.0)

            def wm_chunk(ot, kh):
                ro = slice(ot * P, (ot + 1) * P)
                ks = slice(kh * KC, (kh + 1) * KC)
                wch = wld_pool.tile([P, KC], mybir.dt.float32, tag="wch")
                nc.sync.dma_start(wch[:, :], w[ro, ks])
                mch = mld_pool.tile([P, KC], mybir.dt.int32, tag="mch")
                nc.sync.dma_start(mch[:, :], m[ro, ks])
                wmch = wmbf_pool.tile([P, KC], mybir.dt.bfloat16, tag="wmch")
                nc.vector.tensor_mul(wmch[:, :], wch[:, :], mch[:, :])
                # PE transpose (identity matmul) into resident wmT
                pt = ptr_pool.tile([P, KCT, P], mybir.dt.bfloat16, tag="pt")
                for j in range(KCT):
                    nc.tensor.transpose(pt[:, j, :],
                                        wmch[:, j * P:(j + 1) * P],
                                        ident[:, :])
                nc.vector.tensor_copy(
                    wmT[:, ot, kh * KCT:(kh + 1) * KCT, :], pt[:, :, :])

            def load_panel_bf(tp, split):
                """Plain f32 chunk loads + DVE cast -> xbf bf16 panel."""
                xb = xbf_pool.tile([P, k], mybir.dt.bfloat16, tag="xbf")
                for c in range(XC):
                    ks = slice(c * XKC, (c + 1) * XKC)
                    xc_t = xld_pool.tile([P, XKC], mybir.dt.float32,
                                         tag="xld")
                    eng = nc.sync if (split and c % 2) else nc.scalar
                    eng.dma_start(xc_t[:, :], x[tp * P:(tp + 1) * P, ks])
                    nc.vector.tensor_copy(xb[:, ks], xc_t[:, :])
                return xb

            def prep_panel_xbar(tp):
                xb = load_panel_bf(tp, split=True)
                xt = xTp_pool.tile([P, KT, P], mybir.dt.bfloat16, tag="xT")
                # Single-ring xbar: concurrent DMA-transposes on two HWDGE
                # rings corrupt data on HW (known xbar hazard).
                nc.scalar.dma_start_transpose(xt[:, :, :], xb[:, :])
                return xt

            def prep_panel_pe(tp):
                xb = load_panel_bf(tp, split=False)
                xt = xTp_pool.tile([P, KT, P], mybir.dt.bfloat16, tag="xT")
                for g in range(KT // KCT):
                    pt = ptr_pool.tile([P, KCT, P], mybir.dt.bfloat16,
                                       tag="pt")
                    for j in range(KCT):
                        kt = g * KCT + j
                        nc.tensor.transpose(pt[:, j, :],
                                            xb[:, kt * P:(kt + 1) * P],
                                            ident[:, :])
                    nc.vector.tensor_copy(
                        xt[:, g * KCT:(g + 1) * KCT, :], pt[:, :, :])
                return xt

            panels = {}
            next_prep = 0

            def prep_next():
                nonlocal next_prep
                if next_prep < TP:
                    if next_prep < NPE:
                        panels[next_prep] = prep_panel_pe(next_prep)
                    else:
                        panels[next_prep] = prep_panel_xbar(next_prep)
                    next_prep += 1

            # ---- wm prep: oq-major, kh-major within oq, with PE panel
            # preps interleaved to fill the ramp.
            for oq in range(OQ):
                for kh in range(KH):
                    for ot in range(oq * OTG, (oq + 1) * OTG):
                        wm_chunk(ot, kh)
                    if next_prep < min(R, TP):
                        prep_next()

            while next_prep < min(R, TP):
                prep_next()

            tile_seq = [0]

            def sweep(tp, oq):
                xt = panels[tp]
                t0 = tp * P
                # Explicit round-robin tags: the pool otherwise reuses the
                # most-recently-freed buffer, collapsing the effective
                # pipeline depth to 2 and stalling the PE on evictions.
                j = tile_seq[0]
                tile_seq[0] += 1
                pt = psum_pool.tile([P, OGW], mybir.dt.float32,
                                    tag=f"acc{j % 6}")
                # bias enters as a rank-1 matmul opening the accumulation so
                # the eviction is a pure copy that can run on the ACT engine
                nc.tensor.matmul(
                    pt[:, :], ones_t[:, :],
                    bias_bf[:, oq * OGW:(oq + 1) * OGW],
                    start=True, stop=False)
                for kt in range(KT):
                    nc.tensor.matmul(
                        pt[:, :], xt[:, kt, :],
                        wmT[:, oq * OTG:(oq + 1) * OTG, kt, :],
                        start=False, stop=(kt == KT - 1))
                ot_t = outp.tile([P, OGW], mybir.dt.float32,
                                 tag=f"out{j % 3}")
                # Evict on ACT: on the DVE the evictions sit behind panel
                # casts that wait for load data, which couples the PSUM
                # recycle loop to the slowest chain and stalls the next
                # tile's first MM.
                nc.scalar.copy(ot_t[:, :], pt[:, :])
                nc.sync.dma_start(
                    y[t0:t0 + P, oq * OGW:(oq + 1) * OGW], ot_t[:, :])

            # ---- ramp: R token tiles sweep oq0 first, oq1 deferred
            for tp in range(R):
                sweep(tp, 0)
            for tp in range(R):
                if OQ > 1:
                    sweep(tp, 1)
                prep_next()
                panels.pop(tp)

            # ---- main: sweeps first so evictions never queue behind the
            # next panel's casts in the DVE FIFO.
            for tp in range(R, TP):
                for oq in range(OQ):
                    sweep(tp, oq)
                prep_next()
                panels.pop(tp)

    nc.compile()
    if hw:
        nc.m = get_hw_module(nc.m)
    return nc


_PROGRAM = None


def _get_program():
    global _PROGRAM
    if _PROGRAM is None:
        _PROGRAM = build_program()
    return _PROGRAM


def _enable_tracing():
    """Install the axon NTFF profile hook if the image's antenv lacks it."""
    try:
        import contextlib
        import ctypes
        import types

        import concourse.bass_utils as bu
        bu.upload_artifacts = lambda tmpdir: ""  # no S3 in this container

        try:
            from antenv.axon_hooks import get_axon_ntff_profile_hook
            if get_axon_ntff_profile_hook() is not None:
                return True
        except ImportError:
            pass

        so_path = "/opt/axon/libaxon_pjrt.so"
        if not os.path.exists(so_path):
            return False
        lib = ctypes.CDLL(so_path)
        if not hasattr(lib, "axon_start_nrt_profile"):
            return False
        lib.axon_start_nrt_profile.argtypes = [
            ctypes.POINTER(ctypes.c_int64), ctypes.c_size_t]
        lib.axon_start_nrt_profile.restype = ctypes.c_int64
        lib.axon_stop_nrt_profile.argtypes = [ctypes.c_char_p]
        lib.axon_stop_nrt_profile.restype = ctypes.c_int64

        @contextlib.contextmanager
        def _hook(output_dir, device_ids):
            import jax
            jax.devices()
            if device_ids:
                ids = (ctypes.c_int64 * len(device_ids))(*device_ids)
                rc = lib.axon_start_nrt_profile(ids, len(device_ids))
            else:
                rc = lib.axon_start_nrt_profile(None, 0)
            if rc != 0:
                raise RuntimeError(f"axon_start_nrt_profile rc={rc}")
            try:
                yield
            finally:
                n = lib.axon_stop_nrt_profile(str(output_dir).encode())
                if n <= 0:
                    print(f"ntff profile: rc={n} (no files) -> {output_dir}")

        mod = types.ModuleType("antenv.axon_hooks")
        _state = {"hook": _hook}
        mod.set_axon_ntff_profile_hook = lambda h: _state.update(hook=h)
        mod.get_axon_ntff_profile_hook = lambda: _state["hook"]
        import antenv
        sys.modules["antenv.axon_hooks"] = mod
        antenv.axon_hooks = mod
        return True
    except Exception as e:  # tracing is best-effort
        print(f"tracing unavailable: {e}")
        return False


def kernel(x, weight, bias, mask):
    x = np.asarray(x, dtype=np.float32)
    weight = np.asarray(weight, dtype=np.float32)
    bias = np.asarray(bias, dtype=np.float32)
    mask = np.asarray(mask, dtype=np.int32)

    nc = _get_program()

    in_maps = []
    for core in range(8):
        r, c = core // C_SHARDS, core % C_SHARDS
        in_maps.append({
            "x": np.ascontiguousarray(x[r * T_C:(r + 1) * T_C]),
            "w": np.ascontiguousarray(weight[c * O_C:(c + 1) * O_C]),
            "m": np.ascontiguousarray(mask[c * O_C:(c + 1) * O_C]),
            "b": np.ascontiguousarray(bias[c * O_C:(c + 1) * O_C]),
        })

    trace = os.environ.get("KERNEL_TRACE", "1") == "1"
    if trace:
        trace = _enable_tracing()
    res = None
    if trace:
        tmpdir = os.environ.get("KERNEL_TRACE_DIR")
        if tmpdir:
            os.makedirs(tmpdir, exist_ok=True)
        try:
            res = run_bass_kernel_spmd(nc, in_maps, core_ids=list(range(8)),
                                       trace=True, tmpdir=tmpdir)
        except Exception as e:
            print(f"traced run failed ({e!r}); rerunning untraced")
            res = None
    if res is None:
        res = run_bass_kernel_spmd(nc, in_maps, core_ids=list(range(8)))
    if res.exec_time_ns is not None:
        print(f"HW exec time: {res.exec_time_ns} ns")

    out = np.empty((TOKENS, OUTDIM), dtype=np.float32)
    for core in range(8):
        r, c = core // C_SHARDS, core % C_SHARDS
        out[r * T_C:(r + 1) * T_C, c * O_C:(c + 1) * O_C] = \
            res.results[core]["y"]
    return out


# revision 30
# speedup vs baseline: 1.0010x; 1.0010x over previous
"""ExpanderLinear on 8 TRN2 NeuronCores.

y = x @ (weight * mask)^T + bias
  x      [8192, 4096] f32
  weight [4096, 4096] f32
  mask   [4096, 4096] i32 (0/1)
  bias   [4096]       f32
  y      [8192, 4096] f32

Sharding: 2D 2x4 grid — 2 token shards x 4 outdim shards. Core k handles
tokens [4096*(k//4), +4096) and outdim [1024*(k%4), +1024).

Per-core dataflow (all compute on device). Measured DMA rates drive the
design: plain HWDGE loads ~330GB/s, cast-DMAs only ~230GB/s, xbar
transposes ~130GB/s under load. So:
  - NO cast DMAs: w/m/x load plain (HWDGE rings); DVE does i32->f32
    conversion implicitly in the mask multiply and the x f32->bf16 casts.
  - wm prep: w,m stream on the sync ring; DVE multiply -> bf16; the PE
    transposes wm chunks (identity matmul -> PSUM, DVE copyback) into the
    resident wmT [128, 8ot, 32kt, 128o] bf16 buffer — this rides the
    otherwise-idle ramp phase.
  - x panels [128, 32kt, 128t]: first NPE panels transposed by the PE
    (ramp); the rest via xbar dma_start_transpose (scalar ring) whose
    steady-state demand (~63GB/s) is far below its capability.
  - PE main work: per (tp, oq) a [128t, 512o] PSUM tile accumulates 32
    k-tile matmuls (lhsT = xT panel slice, rhs = 3D wmT slice).
  - DVE eviction adds bias while copying PSUM -> SBUF; y stores on SWDGE.
Ramp: the first R token tiles emit oq0 sweeps before any oq1 work, and wm
chunks stream kh-major, so PSUM accumulation starts k-tile-progressively
as soon as the first quarter of wmT has landed.
"""
import os
import sys

sys.path.insert(0, "/opt/trn_rl_repo")

import numpy as np  # noqa: E402

import concourse.bass as bass  # noqa: E402,F401
import concourse.mybir as mybir  # noqa: E402
import concourse.tile as tile  # noqa: E402
import concourse.bacc as bacc  # noqa: E402
from concourse.bass_utils import run_bass_kernel_spmd  # noqa: E402
from concourse.bass_interp import get_hw_module  # noqa: E402

TOKENS, INDIM, OUTDIM = 8192, 4096, 4096
R_SHARDS, C_SHARDS = 2, 4
T_C, O_C = TOKENS // R_SHARDS, OUTDIM // C_SHARDS  # 4096, 1024

P = 128     # partition / tile edge
OGW = 512   # psum free width (outs per accumulation group)


def build_program(t_c=T_C, o_c=O_C, k=INDIM, hw=True):
    from concourse.masks import make_identity

    TP = t_c // P            # token tiles
    OT = o_c // P            # out tiles
    KT = k // P              # k tiles
    OQ = o_c // OGW          # out groups per token tile
    OTG = OGW // P           # out tiles per group (4)
    KH = 4                   # k chunks for wm staging
    KC = k // KH
    KCT = KC // P            # k tiles per chunk (8)
    XC = 2                   # x load chunks per panel
    XKC = k // XC
    NPAN = 5                 # xT panel bufs
    R = min(4, TP)           # ramp sweeps with deferred oq1
    NPE = min(6, TP)         # panels transposed on the PE during ramp

    nc = bacc.Bacc("TRN2", target_bir_lowering=False, debug=False,
                   num_devices=8)
    x = nc.dram_tensor("x", [t_c, k], mybir.dt.float32, kind="ExternalInput")
    w = nc.dram_tensor("w", [o_c, k], mybir.dt.float32, kind="ExternalInput")
    m = nc.dram_tensor("m", [o_c, k], mybir.dt.int32, kind="ExternalInput")
    b = nc.dram_tensor("b", [o_c], mybir.dt.float32, kind="ExternalInput")
    y = nc.dram_tensor("y", [t_c, o_c], mybir.dt.float32,
                       kind="ExternalOutput")

    with tile.TileContext(nc) as tc:
        with (tc.tile_pool(name="wmT_pool", bufs=1) as wmT_pool,
              tc.tile_pool(name="biasp", bufs=1) as biasp,
              tc.tile_pool(name="consts", bufs=1) as consts,
              tc.tile_pool(name="xTp", bufs=NPAN) as xTp_pool,
              tc.tile_pool(name="wld", bufs=3) as wld_pool,
              tc.tile_pool(name="mld", bufs=3) as mld_pool,
              tc.tile_pool(name="wmbf", bufs=2) as wmbf_pool,
              tc.tile_pool(name="xld", bufs=6) as xld_pool,
              tc.tile_pool(name="xbf", bufs=2) as xbf_pool,
              tc.tile_pool(name="outp", bufs=1) as outp,
              tc.tile_pool(name="psum", bufs=1, space="PSUM") as psum_pool,
              tc.tile_pool(name="ptr", bufs=2, space="PSUM") as ptr_pool):
            wmT = wmT_pool.tile([P, OT, KT, P], mybir.dt.bfloat16, name="wmT")
            bias_bf = biasp.tile([1, o_c], mybir.dt.bfloat16, name="bias")
            nc.gpsimd.dma_start(bias_bf[:, :], b[None, :])
            ident = consts.tile([P, P], mybir.dt.bfloat16, name="ident")
            make_identity(nc, ident)
            ones_t = consts.tile([1, P], mybir.dt.bfloat16, name="ones_t")
            nc.vector.memset(ones_t[:, :], 1.0)

            def wm_chunk(ot, kh):
                ro = slice(ot * P, (ot + 1) * P)
                ks = slice(kh * KC, (kh + 1) * KC)
                wch = wld_pool.tile([P, KC], mybir.dt.float32, tag="wch")
                nc.sync.dma_start(wch[:, :], w[ro, ks])
                mch = mld_pool.tile([P, KC], mybir.dt.int32, tag="mch")
                nc.sync.dma_start(mch[:, :], m[ro, ks])
                wmch = wmbf_pool.tile([P, KC], mybir.dt.bfloat16, tag="wmch")
                nc.vector.tensor_mul(wmch[:, :], wch[:, :], mch[:, :])
                # PE transpose (identity matmul) into resident wmT
                pt = ptr_pool.tile([P, KCT, P], mybir.dt.bfloat16, tag="pt")
                for j in range(KCT):
                    nc.tensor.transpose(pt[:, j, :],
                                        wmch[:, j * P:(j + 1) * P],
                                        ident[:, :])
                nc.vector.tensor_copy(
                    wmT[:, ot, kh * KCT:(kh + 1) * KCT, :], pt[:, :, :])

            def load_panel_bf(tp, split):
                """Plain f32 chunk loads + DVE cast -> xbf bf16 panel."""
                xb = xbf_pool.tile([P, k], mybir.dt.bfloat16, tag="xbf")
                for c in range(XC):
                    ks = slice(c * XKC, (c + 1) * XKC)
                    xc_t = xld_pool.tile([P, XKC], mybir.dt.float32,
                                         tag="xld")
                    eng = nc.sync if (split and c % 2) else nc.scalar
                    eng.dma_start(xc_t[:, :], x[tp * P:(tp + 1) * P, ks])
                    nc.vector.tensor_copy(xb[:, ks], xc_t[:, :])
                return xb

            def prep_panel_xbar(tp):
                xb = load_panel_bf(tp, split=True)
                xt = xTp_pool.tile([P, KT, P], mybir.dt.bfloat16, tag="xT")
                # Single-ring xbar (two concurrent transpose rings corrupt
                # data on HW); on sync so the 4.8us transpose issue spans
                # never delay the ACT eviction copies.
                nc.sync.dma_start_transpose(xt[:, :, :], xb[:, :])
                return xt

            def prep_panel_pe(tp):
                xb = load_panel_bf(tp, split=False)
                xt = xTp_pool.tile([P, KT, P], mybir.dt.bfloat16, tag="xT")
                for g in range(KT // KCT):
                    pt = ptr_pool.tile([P, KCT, P], mybir.dt.bfloat16,
                                       tag="pt")
                    for j in range(KCT):
                        kt = g * KCT + j
                        nc.tensor.transpose(pt[:, j, :],
                                            xb[:, kt * P:(kt + 1) * P],
                                            ident[:, :])
                    nc.vector.tensor_copy(
                        xt[:, g * KCT:(g + 1) * KCT, :], pt[:, :, :])
                return xt

            panels = {}
            next_prep = 0

            def prep_next():
                nonlocal next_prep
                if next_prep < TP:
                    if next_prep < NPE:
                        panels[next_prep] = prep_panel_pe(next_prep)
                    else:
                        panels[next_prep] = prep_panel_xbar(next_prep)
                    next_prep += 1

            # ---- wm prep: oq-major, kh-major within oq, with PE panel
            # preps interleaved to fill the ramp.
            for oq in range(OQ):
                for kh in range(KH):
                    for ot in range(oq * OTG, (oq + 1) * OTG):
                        wm_chunk(ot, kh)
                    if next_prep < min(R, TP):
                        prep_next()

            while next_prep < min(R, TP):
                prep_next()

            tile_seq = [0]

            def sweep(tp, oq):
                xt = panels[tp]
                t0 = tp * P
                # Explicit round-robin tags: the pool otherwise reuses the
                # most-recently-freed buffer, collapsing the effective
                # pipeline depth to 2 and stalling the PE on evictions.
                j = tile_seq[0]
                tile_seq[0] += 1
                pt = psum_pool.tile([P, OGW], mybir.dt.float32,
                                    tag=f"acc{j % 6}")
                # bias enters as a rank-1 matmul opening the accumulation so
                # the eviction is a pure copy that can run on the ACT engine
                nc.tensor.matmul(
                    pt[:, :], ones_t[:, :],
                    bias_bf[:, oq * OGW:(oq + 1) * OGW],
                    start=True, stop=False)
                for kt in range(KT):
                    nc.tensor.matmul(
                        pt[:, :], xt[:, kt, :],
                        wmT[:, oq * OTG:(oq + 1) * OTG, kt, :],
                        start=False, stop=(kt == KT - 1))
                ot_t = outp.tile([P, OGW], mybir.dt.float32,
                                 tag=f"out{j % 3}")
                # Evict on ACT: on the DVE the evictions sit behind panel
                # casts that wait for load data, which couples the PSUM
                # recycle loop to the slowest chain and stalls the next
                # tile's first MM.
                nc.scalar.copy(ot_t[:, :], pt[:, :])
                nc.sync.dma_start(
                    y[t0:t0 + P, oq * OGW:(oq + 1) * OGW], ot_t[:, :])

            # ---- ramp: R token tiles sweep oq0 first, oq1 deferred
            for tp in range(R):
                sweep(tp, 0)
            for tp in range(R):
                if OQ > 1:
                    sweep(tp, 1)
                prep_next()
                panels.pop(tp)

            # ---- main: sweeps first so evictions never queue behind the
            # next panel's casts in the DVE FIFO.
            for tp in range(R, TP):
                for oq in range(OQ):
                    sweep(tp, oq)
                prep_next()
                panels.pop(tp)

    nc.compile()
    if hw:
        nc.m = get_hw_module(nc.m)
    return nc


_PROGRAM = None


def _get_program():
    global _PROGRAM
    if _PROGRAM is None:
        _PROGRAM = build_program()
    return _PROGRAM


def _enable_tracing():
    """Install the axon NTFF profile hook if the image's antenv lacks it."""
    try:
        import contextlib
        import ctypes
        import types

        import concourse.bass_utils as bu
        bu.upload_artifacts = lambda tmpdir: ""  # no S3 in this container

        try:
            from antenv.axon_hooks import get_axon_ntff_profile_hook
            if get_axon_ntff_profile_hook() is not None:
                return True
        except ImportError:
            pass

        so_path = "/opt/axon/libaxon_pjrt.so"
        if not os.path.exists(so_path):
            return False
        lib = ctypes.CDLL(so_path)
        if not hasattr(lib, "axon_start_nrt_profile"):
            return False
        lib.axon_start_nrt_profile.argtypes = [
            ctypes.POINTER(ctypes.c_int64), ctypes.c_size_t]
        lib.axon_start_nrt_profile.restype = ctypes.c_int64
        lib.axon_stop_nrt_profile.argtypes = [ctypes.c_char_p]
        lib.axon_stop_nrt_profile.restype = ctypes.c_int64

        @contextlib.contextmanager
        def _hook(output_dir, device_ids):
            import jax
            jax.devices()
            if device_ids:
                ids = (ctypes.c_int64 * len(device_ids))(*device_ids)
                rc = lib.axon_start_nrt_profile(ids, len(device_ids))
            else:
                rc = lib.axon_start_nrt_profile(None, 0)
            if rc != 0:
                raise RuntimeError(f"axon_start_nrt_profile rc={rc}")
            try:
                yield
            finally:
                n = lib.axon_stop_nrt_profile(str(output_dir).encode())
                if n <= 0:
                    print(f"ntff profile: rc={n} (no files) -> {output_dir}")

        mod = types.ModuleType("antenv.axon_hooks")
        _state = {"hook": _hook}
        mod.set_axon_ntff_profile_hook = lambda h: _state.update(hook=h)
        mod.get_axon_ntff_profile_hook = lambda: _state["hook"]
        import antenv
        sys.modules["antenv.axon_hooks"] = mod
        antenv.axon_hooks = mod
        return True
    except Exception as e:  # tracing is best-effort
        print(f"tracing unavailable: {e}")
        return False


def kernel(x, weight, bias, mask):
    x = np.asarray(x, dtype=np.float32)
    weight = np.asarray(weight, dtype=np.float32)
    bias = np.asarray(bias, dtype=np.float32)
    mask = np.asarray(mask, dtype=np.int32)

    nc = _get_program()

    in_maps = []
    for core in range(8):
        r, c = core // C_SHARDS, core % C_SHARDS
        in_maps.append({
            "x": np.ascontiguousarray(x[r * T_C:(r + 1) * T_C]),
            "w": np.ascontiguousarray(weight[c * O_C:(c + 1) * O_C]),
            "m": np.ascontiguousarray(mask[c * O_C:(c + 1) * O_C]),
            "b": np.ascontiguousarray(bias[c * O_C:(c + 1) * O_C]),
        })

    trace = os.environ.get("KERNEL_TRACE", "1") == "1"
    if trace:
        trace = _enable_tracing()
    res = None
    if trace:
        tmpdir = os.environ.get("KERNEL_TRACE_DIR")
        if tmpdir:
            os.makedirs(tmpdir, exist_ok=True)
        try:
            res = run_bass_kernel_spmd(nc, in_maps, core_ids=list(range(8)),
                                       trace=True, tmpdir=tmpdir)
        except Exception as e:
            print(f"traced run failed ({e!r}); rerunning untraced")
            res = None
    if res is None:
        res = run_bass_kernel_spmd(nc, in_maps, core_ids=list(range(8)))
    if res.exec_time_ns is not None:
        print(f"HW exec time: {res.exec_time_ns} ns")

    out = np.empty((TOKENS, OUTDIM), dtype=np.float32)
    for core in range(8):
        r, c = core // C_SHARDS, core % C_SHARDS
        out[r * T_C:(r + 1) * T_C, c * O_C:(c + 1) * O_C] = \
            res.results[core]["y"]
    return out


# revision 34
# speedup vs baseline: 1.0054x; 1.0044x over previous
"""ExpanderLinear on 8 TRN2 NeuronCores.

y = x @ (weight * mask)^T + bias
  x      [8192, 4096] f32
  weight [4096, 4096] f32
  mask   [4096, 4096] i32 (0/1)
  bias   [4096]       f32
  y      [8192, 4096] f32

Sharding: 2D 2x4 grid — 2 token shards x 4 outdim shards. Core k handles
tokens [4096*(k//4), +4096) and outdim [1024*(k%4), +1024).

Per-core dataflow (all compute on device). Measured DMA rates drive the
design: plain HWDGE loads ~330GB/s, cast-DMAs only ~230GB/s, xbar
transposes ~130GB/s under load. So:
  - NO cast DMAs: w/m/x load plain (HWDGE rings); DVE does i32->f32
    conversion implicitly in the mask multiply and the x f32->bf16 casts.
  - wm prep: w,m stream on the sync ring; DVE multiply -> bf16; the PE
    transposes wm chunks (identity matmul -> PSUM, DVE copyback) into the
    resident wmT [128, 8ot, 32kt, 128o] bf16 buffer — this rides the
    otherwise-idle ramp phase.
  - x panels [128, 32kt, 128t]: first NPE panels transposed by the PE
    (ramp); the rest via xbar dma_start_transpose (scalar ring) whose
    steady-state demand (~63GB/s) is far below its capability.
  - PE main work: per (tp, oq) a [128t, 512o] PSUM tile accumulates 32
    k-tile matmuls (lhsT = xT panel slice, rhs = 3D wmT slice).
  - DVE eviction adds bias while copying PSUM -> SBUF; y stores on SWDGE.
Ramp: the first R token tiles emit oq0 sweeps before any oq1 work, and wm
chunks stream kh-major, so PSUM accumulation starts k-tile-progressively
as soon as the first quarter of wmT has landed.
"""
import os
import sys

sys.path.insert(0, "/opt/trn_rl_repo")

import numpy as np  # noqa: E402

import concourse.bass as bass  # noqa: E402,F401
import concourse.mybir as mybir  # noqa: E402
import concourse.tile as tile  # noqa: E402
import concourse.bacc as bacc  # noqa: E402
from concourse.bass_utils import run_bass_kernel_spmd  # noqa: E402
from concourse.bass_interp import get_hw_module  # noqa: E402

TOKENS, INDIM, OUTDIM = 8192, 4096, 4096
R_SHARDS, C_SHARDS = 2, 4
T_C, O_C = TOKENS // R_SHARDS, OUTDIM // C_SHARDS  # 4096, 1024

P = 128     # partition / tile edge
OGW = 512   # psum free width (outs per accumulation group)


def build_program(t_c=T_C, o_c=O_C, k=INDIM, hw=True):
    from concourse.masks import make_identity

    TP = t_c // P            # token tiles
    OT = o_c // P            # out tiles
    KT = k // P              # k tiles
    OQ = o_c // OGW          # out groups per token tile
    OTG = OGW // P           # out tiles per group (4)
    KH = 4                   # k chunks for wm staging
    KC = k // KH
    KCT = KC // P            # k tiles per chunk (8)
    XC = 2                   # x load chunks per panel
    XKC = k // XC
    NPAN = 6                 # xT panel bufs
    R = min(4, TP)           # ramp sweeps with deferred oq1
    NPE = min(6, TP)         # panels transposed on the PE during ramp

    nc = bacc.Bacc("TRN2", target_bir_lowering=False, debug=False,
                   num_devices=8)
    x = nc.dram_tensor("x", [t_c, k], mybir.dt.float32, kind="ExternalInput")
    w = nc.dram_tensor("w", [o_c, k], mybir.dt.float32, kind="ExternalInput")
    m = nc.dram_tensor("m", [o_c, k], mybir.dt.int32, kind="ExternalInput")
    b = nc.dram_tensor("b", [o_c], mybir.dt.float32, kind="ExternalInput")
    y = nc.dram_tensor("y", [t_c, o_c], mybir.dt.float32,
                       kind="ExternalOutput")

    with tile.TileContext(nc) as tc:
        with (tc.tile_pool(name="wmT_pool", bufs=1) as wmT_pool,
              tc.tile_pool(name="biasp", bufs=1) as biasp,
              tc.tile_pool(name="consts", bufs=1) as consts,
              tc.tile_pool(name="xTp", bufs=NPAN) as xTp_pool,
              tc.tile_pool(name="wld", bufs=3) as wld_pool,
              tc.tile_pool(name="mld", bufs=3) as mld_pool,
              tc.tile_pool(name="wmbf", bufs=2) as wmbf_pool,
              tc.tile_pool(name="xld", bufs=3) as xld_pool,
              tc.tile_pool(name="xbf", bufs=3) as xbf_pool,
              tc.tile_pool(name="outp", bufs=1) as outp,
              tc.tile_pool(name="psum", bufs=1, space="PSUM") as psum_pool,
              tc.tile_pool(name="ptr", bufs=2, space="PSUM") as ptr_pool):
            wmT = wmT_pool.tile([P, OT, KT, P], mybir.dt.bfloat16, name="wmT")
            bias_sb = biasp.tile([P, o_c], mybir.dt.float32, name="bias")
            nc.gpsimd.dma_start(bias_sb[:, :],
                                b[None, :].to_broadcast((P, o_c)))
            ident = consts.tile([P, P], mybir.dt.bfloat16, name="ident")
            make_identity(nc, ident)

            def wm_chunk(ot, kh):
                ro = slice(ot * P, (ot + 1) * P)
                ks = slice(kh * KC, (kh + 1) * KC)
                wch = wld_pool.tile([P, KC], mybir.dt.float32, tag="wch")
                nc.sync.dma_start(wch[:, :], w[ro, ks])
                mch = mld_pool.tile([P, KC], mybir.dt.int32, tag="mch")
                nc.sync.dma_start(mch[:, :], m[ro, ks])
                wmch = wmbf_pool.tile([P, KC], mybir.dt.bfloat16, tag="wmch")
                nc.vector.tensor_mul(wmch[:, :], wch[:, :], mch[:, :])
                # PE transpose (identity matmul) into resident wmT
                pt = ptr_pool.tile([P, KCT, P], mybir.dt.bfloat16, tag="pt")
                for j in range(KCT):
                    nc.tensor.transpose(pt[:, j, :],
                                        wmch[:, j * P:(j + 1) * P],
                                        ident[:, :])
                nc.vector.tensor_copy(
                    wmT[:, ot, kh * KCT:(kh + 1) * KCT, :], pt[:, :, :])

            def load_panel_bf(tp, split):
                """Plain f32 chunk loads + DVE cast -> xbf bf16 panel."""
                xb = xbf_pool.tile([P, k], mybir.dt.bfloat16, tag="xbf")
                for c in range(XC):
                    ks = slice(c * XKC, (c + 1) * XKC)
                    xc_t = xld_pool.tile([P, XKC], mybir.dt.float32,
                                         tag="xld")
                    eng = nc.sync if (split and c % 2) else nc.scalar
                    eng.dma_start(xc_t[:, :], x[tp * P:(tp + 1) * P, ks])
                    nc.vector.tensor_copy(xb[:, ks], xc_t[:, :])
                return xb

            def prep_panel_xbar(tp):
                # Steady-state panels: SWDGE cast-DMA straight to bf16.
                # The gpsimd ring is idle here and its ~230GB/s cast rate
                # covers the ~121GB/s demand; this keeps the x casts out of
                # the DVE FIFO so evictions always fire right after their
                # stop-matmul and PSUM recycling never stalls the PE.
                xb = xbf_pool.tile([P, k], mybir.dt.bfloat16, tag="xbf")
                nc.gpsimd.dma_start(xb[:, :], x[tp * P:(tp + 1) * P, :])
                xt = xTp_pool.tile([P, KT, P], mybir.dt.bfloat16, tag="xT")
                # Single-ring xbar: concurrent DMA-transposes on two HWDGE
                # rings corrupt data on HW (known xbar hazard).
                nc.scalar.dma_start_transpose(xt[:, :, :], xb[:, :])
                return xt

            def prep_panel_pe(tp):
                xb = load_panel_bf(tp, split=False)
                xt = xTp_pool.tile([P, KT, P], mybir.dt.bfloat16, tag="xT")
                for g in range(KT // KCT):
                    pt = ptr_pool.tile([P, KCT, P], mybir.dt.bfloat16,
                                       tag="pt")
                    for j in range(KCT):
                        kt = g * KCT + j
                        nc.tensor.transpose(pt[:, j, :],
                                            xb[:, kt * P:(kt + 1) * P],
                                            ident[:, :])
                    nc.vector.tensor_copy(
                        xt[:, g * KCT:(g + 1) * KCT, :], pt[:, :, :])
                return xt

            panels = {}
            next_prep = 0

            def prep_next():
                nonlocal next_prep
                if next_prep < TP:
                    if next_prep < NPE:
                        panels[next_prep] = prep_panel_pe(next_prep)
                    else:
                        panels[next_prep] = prep_panel_xbar(next_prep)
                    next_prep += 1

            # ---- wm prep: oq-major, kh-major within oq, with PE panel
            # preps interleaved to fill the ramp.
            for oq in range(OQ):
                for kh in range(KH):
                    for ot in range(oq * OTG, (oq + 1) * OTG):
                        wm_chunk(ot, kh)
                    if next_prep < min(R, TP):
                        prep_next()

            while next_prep < min(R, TP):
                prep_next()

            tile_seq = [0]

            def sweep(tp, oq):
                xt = panels[tp]
                t0 = tp * P
                # Explicit round-robin tags: the pool otherwise reuses the
                # most-recently-freed buffer, collapsing the effective
                # pipeline depth to 2 and stalling the PE on evictions.
                j = tile_seq[0]
                tile_seq[0] += 1
                pt = psum_pool.tile([P, OGW], mybir.dt.float32,
                                    tag=f"acc{j % 6}")
                for kt in range(KT):
                    nc.tensor.matmul(
                        pt[:, :], xt[:, kt, :],
                        wmT[:, oq * OTG:(oq + 1) * OTG, kt, :],
                        start=(kt == 0), stop=(kt == KT - 1))
                ot_t = outp.tile([P, OGW], mybir.dt.float32,
                                 tag=f"out{j % 3}")
                # High priority: the eviction must precede the (slack-rich)
                # panel casts in the DVE FIFO, else PSUM recycling stalls
                # the next tile's first matmul.
                with tc.high_priority():
                    nc.vector.tensor_add(
                        ot_t[:, :], pt[:, :],
                        bias_sb[:, oq * OGW:(oq + 1) * OGW])
                nc.sync.dma_start(
                    y[t0:t0 + P, oq * OGW:(oq + 1) * OGW], ot_t[:, :])

            # ---- ramp: R token tiles sweep oq0 first, oq1 deferred
            for tp in range(R):
                sweep(tp, 0)
            for tp in range(R):
                if OQ > 1:
                    sweep(tp, 1)
                prep_next()
                panels.pop(tp)

            # ---- main: sweeps first so evictions never queue behind the
            # next panel's casts in the DVE FIFO.
            for tp in range(R, TP):
                for oq in range(OQ):
                    sweep(tp, oq)
                prep_next()
                panels.pop(tp)

    nc.compile()
    if hw:
        nc.m = get_hw_module(nc.m)
    return nc


_PROGRAM = None


def _get_program():
    global _PROGRAM
    if _PROGRAM is None:
        _PROGRAM = build_program()
    return _PROGRAM


def _enable_tracing():
    """Install the axon NTFF profile hook if the image's antenv lacks it."""
    try:
        import contextlib
        import ctypes
        import types

        import concourse.bass_utils as bu
        bu.upload_artifacts = lambda tmpdir: ""  # no S3 in this container

        try:
            from antenv.axon_hooks import get_axon_ntff_profile_hook
            if get_axon_ntff_profile_hook() is not None:
                return True
        except ImportError:
            pass

        so_path = "/opt/axon/libaxon_pjrt.so"
        if not os.path.exists(so_path):
            return False
        lib = ctypes.CDLL(so_path)
        if not hasattr(lib, "axon_start_nrt_profile"):
            return False
        lib.axon_start_nrt_profile.argtypes = [
            ctypes.POINTER(ctypes.c_int64), ctypes.c_size_t]
        lib.axon_start_nrt_profile.restype = ctypes.c_int64
        lib.axon_stop_nrt_profile.argtypes = [ctypes.c_char_p]
        lib.axon_stop_nrt_profile.restype = ctypes.c_int64

        @contextlib.contextmanager
        def _hook(output_dir, device_ids):
            import jax
            jax.devices()
            if device_ids:
                ids = (ctypes.c_int64 * len(device_ids))(*device_ids)
                rc = lib.axon_start_nrt_profile(ids, len(device_ids))
            else:
                rc = lib.axon_start_nrt_profile(None, 0)
            if rc != 0:
                raise RuntimeError(f"axon_start_nrt_profile rc={rc}")
            try:
                yield
            finally:
                n = lib.axon_stop_nrt_profile(str(output_dir).encode())
                if n <= 0:
                    print(f"ntff profile: rc={n} (no files) -> {output_dir}")

        mod = types.ModuleType("antenv.axon_hooks")
        _state = {"hook": _hook}
        mod.set_axon_ntff_profile_hook = lambda h: _state.update(hook=h)
        mod.get_axon_ntff_profile_hook = lambda: _state["hook"]
        import antenv
        sys.modules["antenv.axon_hooks"] = mod
        antenv.axon_hooks = mod
        return True
    except Exception as e:  # tracing is best-effort
        print(f"tracing unavailable: {e}")
        return False


def kernel(x, weight, bias, mask):
    x = np.asarray(x, dtype=np.float32)
    weight = np.asarray(weight, dtype=np.float32)
    bias = np.asarray(bias, dtype=np.float32)
    mask = np.asarray(mask, dtype=np.int32)

    nc = _get_program()

    in_maps = []
    for core in range(8):
        r, c = core // C_SHARDS, core % C_SHARDS
        in_maps.append({
            "x": np.ascontiguousarray(x[r * T_C:(r + 1) * T_C]),
            "w": np.ascontiguousarray(weight[c * O_C:(c + 1) * O_C]),
            "m": np.ascontiguousarray(mask[c * O_C:(c + 1) * O_C]),
            "b": np.ascontiguousarray(bias[c * O_C:(c + 1) * O_C]),
        })

    trace = os.environ.get("KERNEL_TRACE", "1") == "1"
    if trace:
        trace = _enable_tracing()
    res = None
    if trace:
        tmpdir = os.environ.get("KERNEL_TRACE_DIR")
        if tmpdir:
            os.makedirs(tmpdir, exist_ok=True)
        try:
            res = run_bass_kernel_spmd(nc, in_maps, core_ids=list(range(8)),
                                       trace=True, tmpdir=tmpdir)
        except Exception as e:
            print(f"traced run failed ({e!r}); rerunning untraced")
            res = None
    if res is None:
        res = run_bass_kernel_spmd(nc, in_maps, core_ids=list(range(8)))
    if res.exec_time_ns is not None:
        print(f"HW exec time: {res.exec_time_ns} ns")

    out = np.empty((TOKENS, OUTDIM), dtype=np.float32)
    for core in range(8):
        r, c = core // C_SHARDS, core % C_SHARDS
        out[r * T_C:(r + 1) * T_C, c * O_C:(c + 1) * O_C] = \
            res.results[core]["y"]
    return out
